# revision 13
# baseline (speedup 1.0000x reference)
"""Trainium2 Bass kernel for Aimv2VisionEmbeddings (patch-embed GEMM + RMSNorm
+ ragged 2D sincos positional embedding), data-parallel over 8 NeuronCores.

Contract: kernel(**inputs) takes the FULL unsharded inputs and returns the
FULL [16, 4096, 1024] float32 output. Internally:
  - batch is sharded 2-per-core across 8 cores,
  - hidden_states is host-cast to bf16 and K-padded 588->640 so the device
    can DMA-transpose (xbar needs 2-byte dtype, 128-col multiples),
  - patch_weight is host-transposed to [K, H] bf16 (tiny),
  - the sincos positional embedding is a gather: pos row for patch n is
    [sin(j*omega)|cos(j*omega)|sin(i*omega)|cos(i*omega)] with j = n mod w,
    i = n // w, and j,i < 64 - so a universal (input-independent) table
    T[v] = [sin(v*omega)|cos(v*omega)] of 64 rows covers every case. The
    per-row indices (from the 16x2 spatial_shapes) ship as an int32 input
    and drive per-tile indirect-DMA gathers on GpSimd.

Device program per core (rows = 2*4096):
  GEMM on TensorE: out[r,:] accumulated in PSUM over 5 K-chunks of 128,
  per 128-row tile (lhsT = DMA-transposed X chunk, rhs = W chunk).
  Sum-of-squares on ScalarE (Square + free-dim accum; the only ACT table
  set used, so no LUT swaps), rstd = rsqrt(ssq/H + eps) on VectorE via a
  bitcast-seed + 2 Newton steps, batched over groups of 3 tiles (PSUM
  holds 3 in-flight tiles + 1 spare). Final fused (x * rstd) + pos is a
  single scalar_tensor_tensor pass straight out of PSUM, written bf16.
"""

import numpy as np
import ml_dtypes

import concourse.bass as bass
import concourse.bacc as bacc
import concourse.mybir as mybir
from concourse import tile
from concourse.bass_utils import run_bass_kernel_spmd

AF = mybir.ActivationFunctionType
ALU = mybir.AluOpType
DT = mybir.dt

B, N, D, H = 16, 4096, 588, 1024
NCORES = 8
LB = B // NCORES          # local batches per core
KP, NK = 640, 5           # zero-padded contraction dim, 5 chunks of 128
POS_DIM = H // 4          # 256
EPS = 1e-6
TEMP = 10000.0
QUAKE_C = 0x5F3759DF


def build(rows_per_b=N, rb=2048, with_bias=False, with_rmsw=False, tsz=64,
          psum_bufs=4, xt_bufs=2, work_bufs=4, grp=3, out_bf16=True):
    """Build the per-core bass program. rows_per_b/rb are shrinkable for sim."""
    rows = LB * rows_per_b
    rb = min(rb, rows_per_b)
    assert rows_per_b % rb == 0 and rb % 128 == 0
    out_dt = DT.bfloat16 if out_bf16 else DT.float32

    nc = bacc.Bacc("TRN2", target_bir_lowering=False, debug=False)
    x_d = nc.declare_dram_parameter("x", [rows, KP], DT.bfloat16, isOutput=False)
    w_d = nc.declare_dram_parameter("w", [KP, H], DT.bfloat16, isOutput=False)
    ij_d = nc.declare_dram_parameter("ij", [rows, 2], DT.int32, isOutput=False)
    t_d = nc.declare_dram_parameter("tbl", [tsz, 512], DT.bfloat16, isOutput=False)
    if with_bias:
        bias_d = nc.declare_dram_parameter("bias", [128, H], DT.float32, isOutput=False)
    if with_rmsw:
        rw_d = nc.declare_dram_parameter("rw", [128, H], DT.float32, isOutput=False)
    out_d = nc.declare_dram_parameter("out", [rows, H], out_dt, isOutput=True)

    with tile.TileContext(nc) as tc:
        with (
            tc.tile_pool(name="const", bufs=1) as cpool,
            tc.tile_pool(name="xt", bufs=xt_bufs) as xpool,
            tc.tile_pool(name="work", bufs=work_bufs) as wpool,
            tc.tile_pool(name="psum", bufs=psum_bufs, space=bass.MemorySpace.PSUM) as ppool,
        ):
            wt = cpool.tile([128, NK, H], DT.bfloat16)
            nc.sync.dma_start(wt[:], w_d.rearrange("(k p) h -> p k h", p=128))
            cq = cpool.tile([128, grp], DT.int32)
            nc.vector.memset(cq[:], QUAKE_C)
            if with_bias:
                biast = cpool.tile([128, H], DT.float32)
                nc.sync.dma_start(biast[:], bias_d[:])
            if with_rmsw:
                rwt = cpool.tile([128, H], DT.float32)
                nc.sync.dma_start(rwt[:], rw_d[:])

            n_blocks = rows // rb
            tiles_per_blk = rb // 128
            for blk in range(n_blocks):
                r0 = blk * rb
                xts = []
                for k in range(NK):
                    xt_k = xpool.tile([128, rb], DT.bfloat16, tag=f"xt{k}")
                    nc.sync.dma_start_transpose(
                        xt_k[:], x_d[r0:r0 + rb, k * 128:(k + 1) * 128]
                    )
                    xts.append(xt_k)
                ijb = xpool.tile([128, tiles_per_blk, 2], DT.int32, tag="ijb")
                nc.sync.dma_start(
                    ijb[:], ij_d[r0:r0 + rb, :].rearrange("(t p) c -> p t c", p=128)
                )

                it = 0
                while it < tiles_per_blk:
                    g = min(grp, tiles_per_blk - it)
                    ssqg = wpool.tile([128, grp], DT.float32, tag="ssqg")
                    xsrcs, poss = [], []
                    for gi in range(g):
                        t = it + gi
                        xacc = ppool.tile([128, H], DT.float32, tag="xacc")
                        for half in range(2):
                            for k in range(NK):
                                nc.tensor.matmul(
                                    xacc[:, half * 512:(half + 1) * 512],
                                    xts[k][:, t * 128:(t + 1) * 128],
                                    wt[:, k, half * 512:(half + 1) * 512],
                                    start=(k == 0),
                                    stop=(k == NK - 1),
                                )

                        if with_bias:
                            xsrc = wpool.tile([128, H], DT.float32, tag="xb")
                            nc.vector.tensor_add(xsrc[:], xacc[:], biast[:])
                        else:
                            xsrc = xacc
                        xsrcs.append(xsrc)

                        # gather pos rows from the sincos table (GpSimd SWDGE)
                        pos = wpool.tile([128, H], DT.bfloat16, tag="pos")
                        poss.append(pos)
                        nc.gpsimd.indirect_dma_start(
                            out=pos[:, 0:512], out_offset=None, in_=t_d[:],
                            in_offset=bass.IndirectOffsetOnAxis(ap=ijb[:, t, 0:1], axis=0),
                        )
                        nc.gpsimd.indirect_dma_start(
                            out=pos[:, 512:1024], out_offset=None, in_=t_d[:],
                            in_offset=bass.IndirectOffsetOnAxis(ap=ijb[:, t, 1:2], axis=0),
                        )

                        # sum of squares for this tile -> ssqg[:, gi]
                        sqd = wpool.tile([128, H], DT.float32, tag="sqd")
                        nc.scalar.activation(
                            sqd[:], xsrc[:], AF.Square, accum_out=ssqg[:, gi:gi + 1]
                        )

                    # rstd = rsqrt(ssq/H + eps) for the whole group on DVE
                    # (bitcast seed + 2 Newton steps; no ACT Sqrt table).
                    gs = slice(0, g)
                    vq = wpool.tile([128, grp], DT.float32, tag="vq")
                    nc.vector.tensor_scalar(vq[:, gs], ssqg[:, gs], 1.0 / H, EPS, ALU.mult, ALU.add)
                    ish = wpool.tile([128, grp], DT.int32, tag="ish")
                    nc.vector.tensor_scalar(
                        ish[:, gs], vq[:, gs].bitcast(DT.int32), 1, None, ALU.arith_shift_right
                    )
                    y0 = wpool.tile([128, grp], DT.int32, tag="y0")
                    nc.vector.tensor_sub(y0[:, gs], cq[:, gs], ish[:, gs])
                    y0f = y0[:, gs].bitcast(DT.float32)
                    qa = wpool.tile([128, grp], DT.float32, tag="qa")
                    nc.vector.tensor_mul(qa[:, gs], y0f, y0f)
                    nc.vector.tensor_mul(qa[:, gs], qa[:, gs], vq[:, gs])
                    nc.vector.tensor_scalar(qa[:, gs], qa[:, gs], -0.5, 1.5, ALU.mult, ALU.add)
                    qy = wpool.tile([128, grp], DT.float32, tag="qy")
                    nc.vector.tensor_mul(qy[:, gs], y0f, qa[:, gs])
                    qb = wpool.tile([128, grp], DT.float32, tag="qb")
                    nc.vector.tensor_mul(qb[:, gs], qy[:, gs], qy[:, gs])
                    nc.vector.tensor_mul(qb[:, gs], qb[:, gs], vq[:, gs])
                    nc.vector.tensor_scalar(qb[:, gs], qb[:, gs], -0.5, 1.5, ALU.mult, ALU.add)
                    rstdg = wpool.tile([128, grp], DT.float32, tag="rstdg")
                    nc.vector.tensor_mul(rstdg[:, gs], qy[:, gs], qb[:, gs])

                    for gi in range(g):
                        t = it + gi
                        row0 = r0 + t * 128
                        rs = rstdg[:, gi:gi + 1]
                        outt = wpool.tile([128, H], out_dt, tag="outt")
                        if with_rmsw:
                            xn = wpool.tile([128, H], DT.float32, tag="xn")
                            nc.vector.tensor_scalar(xn[:], xsrcs[gi][:], rs, None, ALU.mult)
                            nc.vector.tensor_mul(xn[:], xn[:], rwt[:])
                            nc.vector.tensor_add(outt[:], xn[:], poss[gi][:])
                        else:
                            nc.vector.scalar_tensor_tensor(
                                outt[:], xsrcs[gi][:], rs, poss[gi][:], ALU.mult, ALU.add
                            )
                        nc.scalar.dma_start(out_d[row0:row0 + 128, :], outt[:])
                    it += g

    nc.compile()
    return nc


def make_inputs(hidden_states, spatial_shapes, patch_weight, patch_bias,
                rms_weight, rows_per_b=N):
    """Host-side marshalling: shard + cast + pad. Returns (in_maps, meta)."""
    hs = np.asarray(hidden_states, dtype=np.float32)
    ss = np.asarray(spatial_shapes)
    pw = np.asarray(patch_weight, dtype=np.float32).reshape(H, D)
    pb = np.asarray(patch_bias, dtype=np.float32)
    rw = np.asarray(rms_weight, dtype=np.float32)
    with_bias = bool(np.any(pb != 0.0))
    with_rmsw = bool(np.any(rw != 1.0))

    bf16 = ml_dtypes.bfloat16
    hsv = hs[:, :rows_per_b, :]          # [B, rows_per_b, D]
    xp = np.zeros((B * rows_per_b, KP), dtype=bf16)
    xp[:, :D] = hsv.reshape(B * rows_per_b, D).astype(bf16)
    wp = np.zeros((KP, H), dtype=bf16)
    wp[:D, :] = pw.T.astype(bf16)

    # per-row (j, i) indices, pre-masked (invalid rows -> 0), int32
    n = np.arange(rows_per_b, dtype=np.int64)[None, :]       # [1, R]
    hcol = ss[:, 0:1].astype(np.int64)
    wcol = ss[:, 1:2].astype(np.int64)
    valid = n < hcol * wcol
    jv = np.where(valid, n % wcol, 0)
    iv = np.where(valid, n // wcol, 0)
    ij = np.stack([jv, iv], axis=-1).reshape(B * rows_per_b, 2)
    ij = np.ascontiguousarray(ij, dtype=np.int32)

    # universal sincos table: T[v] = [sin(v*omega) | cos(v*omega)]
    tsz = int(max(64, jv.max() + 1, iv.max() + 1))
    om = (1.0 / (TEMP ** (np.arange(POS_DIM, dtype=np.float64) / POS_DIM)))
    ang = np.arange(tsz, dtype=np.float64)[:, None] * om[None, :]
    tbl = np.concatenate([np.sin(ang), np.cos(ang)], axis=1).astype(bf16)
    tbl = np.ascontiguousarray(tbl)

    rows = LB * rows_per_b
    in_maps = []
    for c in range(NCORES):
        m = {
            "x": xp[c * rows:(c + 1) * rows],
            "w": wp,
            "ij": ij[c * rows:(c + 1) * rows],
            "tbl": tbl,
        }
        if with_bias:
            m["bias"] = np.ascontiguousarray(np.broadcast_to(pb, (128, H)))
        if with_rmsw:
            m["rw"] = np.ascontiguousarray(np.broadcast_to(rw, (128, H)))
        in_maps.append(m)
    return in_maps, with_bias, with_rmsw, tsz


_BUILD_CACHE = {}


def kernel(hidden_states, spatial_shapes, patch_weight, patch_bias, rms_weight,
           _trace=False):
    in_maps, with_bias, with_rmsw, tsz = make_inputs(
        hidden_states, spatial_shapes, patch_weight, patch_bias, rms_weight
    )
    key = (with_bias, with_rmsw, tsz)
    if key not in _BUILD_CACHE:
        _BUILD_CACHE[key] = build(with_bias=with_bias, with_rmsw=with_rmsw, tsz=tsz)
    nc = _BUILD_CACHE[key]
    res = run_bass_kernel_spmd(nc, in_maps, list(range(NCORES)), trace=_trace)
    out = np.concatenate([r["out"] for r in res.results], axis=0)
    out = out.reshape(B, N, H).astype(np.float32)
    if _trace:
        kernel.last_results = res
    return out


# revision 14
# speedup vs baseline: 1.2221x; 1.2221x over previous
"""Trainium2 Bass kernel for Aimv2VisionEmbeddings (patch-embed GEMM + RMSNorm
+ ragged 2D sincos positional embedding), data-parallel over 8 NeuronCores.

Contract: kernel(**inputs) takes the FULL unsharded inputs and returns the
FULL [16, 4096, 1024] float32 output. Internally:
  - batch is sharded 2-per-core across 8 cores,
  - hidden_states is host-cast to bf16 and K-padded 588->640 so the device
    can DMA-transpose (xbar needs 2-byte dtype, 128-col multiples),
  - patch_weight is host-transposed to [K, H] bf16 (tiny),
  - the sincos positional embedding is a gather: pos row for patch n is
    [sin(j*omega)|cos(j*omega)|sin(i*omega)|cos(i*omega)] with j = n mod w,
    i = n // w, and j,i < 64 - so a universal (input-independent) table
    T[v] = [sin(v*omega)|cos(v*omega)] of 64 rows covers every case. The
    per-row indices (from the 16x2 spatial_shapes) ship as an int32 input
    and drive per-tile indirect-DMA gathers on GpSimd.

Device program per core (rows = 2*4096):
  GEMM on TensorE: out[r,:] accumulated in PSUM over 5 K-chunks of 128,
  per 128-row tile (lhsT = DMA-transposed X chunk, rhs = W chunk).
  Sum-of-squares on ScalarE (Square + free-dim accum; the only ACT table
  set used, so no LUT swaps), rstd = rsqrt(ssq/H + eps) on VectorE via a
  bitcast-seed + 2 Newton steps, batched over groups of 3 tiles (PSUM
  holds 3 in-flight tiles + 1 spare). Final fused (x * rstd) + pos is a
  single scalar_tensor_tensor pass straight out of PSUM, written bf16.
"""

import numpy as np
import ml_dtypes

import concourse.bass as bass
import concourse.bacc as bacc
import concourse.mybir as mybir
from concourse import tile
from concourse.bass_utils import run_bass_kernel_spmd

AF = mybir.ActivationFunctionType
ALU = mybir.AluOpType
DT = mybir.dt

B, N, D, H = 16, 4096, 588, 1024
NCORES = 8
LB = B // NCORES          # local batches per core
KP, NK = 640, 5           # zero-padded contraction dim, 5 chunks of 128
POS_DIM = H // 4          # 256
EPS = 1e-6
TEMP = 10000.0
QUAKE_C = 0x5F3759DF


def build(rows_per_b=N, rb=2048, with_bias=False, with_rmsw=False, tsz=64,
          psum_bufs=4, xt_bufs=2, work_bufs=4, grp=2, out_bf16=True):
    """Build the per-core bass program. rows_per_b/rb are shrinkable for sim."""
    rows = LB * rows_per_b
    rb = min(rb, rows_per_b)
    assert rows_per_b % rb == 0 and rb % 128 == 0
    out_dt = DT.bfloat16 if out_bf16 else DT.float32

    nc = bacc.Bacc("TRN2", target_bir_lowering=False, debug=False)
    x_d = nc.declare_dram_parameter("x", [rows, KP], DT.bfloat16, isOutput=False)
    w_d = nc.declare_dram_parameter("w", [KP, H], DT.bfloat16, isOutput=False)
    ij_d = nc.declare_dram_parameter("ij", [rows, 1], DT.int32, isOutput=False)
    t_d = nc.declare_dram_parameter("tbl", [tsz * tsz, H], DT.bfloat16, isOutput=False)
    if with_bias:
        bias_d = nc.declare_dram_parameter("bias", [128, H], DT.float32, isOutput=False)
    if with_rmsw:
        rw_d = nc.declare_dram_parameter("rw", [128, H], DT.float32, isOutput=False)
    out_d = nc.declare_dram_parameter("out", [rows, H], out_dt, isOutput=True)

    with tile.TileContext(nc) as tc:
        with (
            tc.tile_pool(name="const", bufs=1) as cpool,
            tc.tile_pool(name="xt", bufs=xt_bufs) as xpool,
            tc.tile_pool(name="work", bufs=work_bufs) as wpool,
            tc.tile_pool(name="psum", bufs=psum_bufs, space=bass.MemorySpace.PSUM) as ppool,
        ):
            wt = cpool.tile([128, NK, H], DT.bfloat16)
            nc.sync.dma_start(wt[:], w_d.rearrange("(k p) h -> p k h", p=128))
            cq = cpool.tile([128, grp], DT.int32)
            nc.vector.memset(cq[:], QUAKE_C)
            if with_bias:
                biast = cpool.tile([128, H], DT.float32)
                nc.sync.dma_start(biast[:], bias_d[:])
            if with_rmsw:
                rwt = cpool.tile([128, H], DT.float32)
                nc.sync.dma_start(rwt[:], rw_d[:])

            n_blocks = rows // rb
            tiles_per_blk = rb // 128
            for blk in range(n_blocks):
                r0 = blk * rb
                xts = []
                for k in range(NK):
                    xt_k = xpool.tile([128, rb], DT.bfloat16, tag=f"xt{k}")
                    nc.sync.dma_start_transpose(
                        xt_k[:], x_d[r0:r0 + rb, k * 128:(k + 1) * 128]
                    )
                    xts.append(xt_k)
                ijb = xpool.tile([128, tiles_per_blk, 1], DT.int32, tag="ijb")
                nc.sync.dma_start(
                    ijb[:], ij_d[r0:r0 + rb, :].rearrange("(t p) c -> p t c", p=128)
                )

                it = 0
                while it < tiles_per_blk:
                    g = min(grp, tiles_per_blk - it)
                    ssqg = wpool.tile([128, grp], DT.float32, tag="ssqg")
                    xsrcs, poss = [], []
                    for gi in range(g):
                        t = it + gi
                        xacc = ppool.tile([128, H], DT.float32, tag="xacc")
                        for half in range(2):
                            for k in range(NK):
                                nc.tensor.matmul(
                                    xacc[:, half * 512:(half + 1) * 512],
                                    xts[k][:, t * 128:(t + 1) * 128],
                                    wt[:, k, half * 512:(half + 1) * 512],
                                    start=(k == 0),
                                    stop=(k == NK - 1),
                                )

                        if with_bias:
                            xsrc = wpool.tile([128, H], DT.float32, tag="xb")
                            nc.vector.tensor_add(xsrc[:], xacc[:], biast[:])
                        else:
                            xsrc = xacc
                        xsrcs.append(xsrc)

                        # gather pos rows from the sincos table (GpSimd SWDGE)
                        pos = wpool.tile([128, H], DT.bfloat16, tag="pos")
                        poss.append(pos)
                        nc.gpsimd.indirect_dma_start(
                            out=pos[:], out_offset=None, in_=t_d[:],
                            in_offset=bass.IndirectOffsetOnAxis(ap=ijb[:, t, 0:1], axis=0),
                        )

                        # sum of squares for this tile -> ssqg[:, gi]
                        sqd = wpool.tile([128, H], DT.float32, tag="sqd")
                        nc.scalar.activation(
                            sqd[:], xsrc[:], AF.Square, accum_out=ssqg[:, gi:gi + 1]
                        )

                    # rstd = rsqrt(ssq/H + eps) for the whole group on DVE
                    # (bitcast seed + 2 Newton steps; no ACT Sqrt table).
                    gs = slice(0, g)
                    vq = wpool.tile([128, grp], DT.float32, tag="vq")
                    nc.vector.tensor_scalar(vq[:, gs], ssqg[:, gs], 1.0 / H, EPS, ALU.mult, ALU.add)
                    ish = wpool.tile([128, grp], DT.int32, tag="ish")
                    nc.vector.tensor_scalar(
                        ish[:, gs], vq[:, gs].bitcast(DT.int32), 1, None, ALU.arith_shift_right
                    )
                    y0 = wpool.tile([128, grp], DT.int32, tag="y0")
                    nc.vector.tensor_sub(y0[:, gs], cq[:, gs], ish[:, gs])
                    y0f = y0[:, gs].bitcast(DT.float32)
                    qa = wpool.tile([128, grp], DT.float32, tag="qa")
                    nc.vector.tensor_mul(qa[:, gs], y0f, y0f)
                    nc.vector.tensor_mul(qa[:, gs], qa[:, gs], vq[:, gs])
                    nc.vector.tensor_scalar(qa[:, gs], qa[:, gs], -0.5, 1.5, ALU.mult, ALU.add)
                    rstdg = wpool.tile([128, grp], DT.float32, tag="rstdg")
                    nc.vector.tensor_mul(rstdg[:, gs], y0f, qa[:, gs])

                    for gi in range(g):
                        t = it + gi
                        row0 = r0 + t * 128
                        rs = rstdg[:, gi:gi + 1]
                        outt = wpool.tile([128, H], out_dt, tag="outt")
                        if with_rmsw:
                            xn = wpool.tile([128, H], DT.float32, tag="xn")
                            nc.vector.tensor_scalar(xn[:], xsrcs[gi][:], rs, None, ALU.mult)
                            nc.vector.tensor_mul(xn[:], xn[:], rwt[:])
                            nc.vector.tensor_add(outt[:], xn[:], poss[gi][:])
                        else:
                            nc.vector.scalar_tensor_tensor(
                                outt[:], xsrcs[gi][:], rs, poss[gi][:], ALU.mult, ALU.add
                            )
                        nc.scalar.dma_start(out_d[row0:row0 + 128, :], outt[:])
                    it += g

    nc.compile()
    return nc


def make_inputs(hidden_states, spatial_shapes, patch_weight, patch_bias,
                rms_weight, rows_per_b=N):
    """Host-side marshalling: shard + cast + pad. Returns (in_maps, meta)."""
    hs = np.asarray(hidden_states, dtype=np.float32)
    ss = np.asarray(spatial_shapes)
    pw = np.asarray(patch_weight, dtype=np.float32).reshape(H, D)
    pb = np.asarray(patch_bias, dtype=np.float32)
    rw = np.asarray(rms_weight, dtype=np.float32)
    with_bias = bool(np.any(pb != 0.0))
    with_rmsw = bool(np.any(rw != 1.0))

    bf16 = ml_dtypes.bfloat16
    hsv = hs[:, :rows_per_b, :]          # [B, rows_per_b, D]
    xp = np.zeros((B * rows_per_b, KP), dtype=bf16)
    xp[:, :D] = hsv.reshape(B * rows_per_b, D).astype(bf16)
    wp = np.zeros((KP, H), dtype=bf16)
    wp[:D, :] = pw.T.astype(bf16)

    # per-row (j, i) indices, pre-masked (invalid rows -> 0), int32
    n = np.arange(rows_per_b, dtype=np.int64)[None, :]       # [1, R]
    hcol = ss[:, 0:1].astype(np.int64)
    wcol = ss[:, 1:2].astype(np.int64)
    valid = n < hcol * wcol
    jv = np.where(valid, n % wcol, 0)
    iv = np.where(valid, n // wcol, 0)
    tsz = int(max(64, jv.max() + 1, iv.max() + 1))
    ij = (jv * tsz + iv).reshape(B * rows_per_b, 1)
    ij = np.ascontiguousarray(ij, dtype=np.int32)

    # universal sincos product table:
    # T[j*tsz+i] = [sin(j*om) | cos(j*om) | sin(i*om) | cos(i*om)]
    om = (1.0 / (TEMP ** (np.arange(POS_DIM, dtype=np.float64) / POS_DIM)))
    ang = np.arange(tsz, dtype=np.float64)[:, None] * om[None, :]
    sc = np.concatenate([np.sin(ang), np.cos(ang)], axis=1).astype(np.float32)  # [tsz, 512]
    tbl = np.empty((tsz, tsz, H), dtype=bf16)
    tbl[:, :, 0:512] = sc[:, None, :]
    tbl[:, :, 512:1024] = sc[None, :, :]
    tbl = np.ascontiguousarray(tbl.reshape(tsz * tsz, H))

    rows = LB * rows_per_b
    in_maps = []
    for c in range(NCORES):
        m = {
            "x": xp[c * rows:(c + 1) * rows],
            "w": wp,
            "ij": ij[c * rows:(c + 1) * rows],
            "tbl": tbl,
        }
        if with_bias:
            m["bias"] = np.ascontiguousarray(np.broadcast_to(pb, (128, H)))
        if with_rmsw:
            m["rw"] = np.ascontiguousarray(np.broadcast_to(rw, (128, H)))
        in_maps.append(m)
    return in_maps, with_bias, with_rmsw, tsz


_BUILD_CACHE = {}


def kernel(hidden_states, spatial_shapes, patch_weight, patch_bias, rms_weight,
           _trace=False):
    in_maps, with_bias, with_rmsw, tsz = make_inputs(
        hidden_states, spatial_shapes, patch_weight, patch_bias, rms_weight
    )
    key = (with_bias, with_rmsw, tsz)
    if key not in _BUILD_CACHE:
        _BUILD_CACHE[key] = build(with_bias=with_bias, with_rmsw=with_rmsw, tsz=tsz)
    nc = _BUILD_CACHE[key]
    res = run_bass_kernel_spmd(nc, in_maps, list(range(NCORES)), trace=_trace)
    out = np.concatenate([r["out"] for r in res.results], axis=0)
    out = out.reshape(B, N, H).astype(np.float32)
    if _trace:
        kernel.last_results = res
    return out
